# revision 1
# baseline (speedup 1.0000x reference)
"""Trainium2 Bass kernel for nn_AlignmentLoss (triplet + CE over phrase/input embeddings).

Sharding: batch dimension N=128 split 16 batches/core across 8 cores.  Each core
owns the positive pairs whose batch_idxs falls in its range (host buckets pairs,
padded to a fixed per-batch capacity cap=64; 2 batches share a 128-partition tile).

v4 design:
 - Host supplies every gathered operand bf16, pre-transposed or pre-packed in
   the exact SBUF tile layout, so the whole prologue is 8 large contiguous
   DMAs (no f32 transposes, casts, or descriptor-bound small DMAs).
 - Input norms^2 come from all-ones-stationary matmuls over squared xt chunks,
   written REPLICATED across the partition halves of a PSUM tile, so one ACT
   Abs_reciprocal_sqrt per batch-pair produces the broadcast inverse-norm tile
   directly (no DRAM round-trip).
 - All per-row norms (anc/pos/rng/phr) are DVE scalar_tensor_tensor
   accumulates + two ACT rsqrts; phase ordering keeps the ACT table on the
   rsqrt set for all of phase B, then switches once to the exp set for the CE
   Exps (a bypass-dependency on the last rnb tile stops the scheduler from
   hoisting Exps between rsqrts).
 - CE logits use RAW posT as the matmul stationary and fold T*rinv_pos into
   the per-partition scale of the fused Exp+accum.
 - Rows are scaled by (rinv_anchor per-partition) * (rnb tensor) in one DVE
   scalar_tensor_tensor from PSUM, then a single DVE max8 per batch-pair.
"""

import sys

for _p in ("/opt/trn_rl_repo", "/root/.axon_site/_ro/trn_rl_repo"):
    if _p not in sys.path:
        sys.path.append(_p)

import numpy as np

import concourse.bass as bass
import concourse.bacc as bacc
import concourse.mybir as mybir
from concourse.tile import TileContext
from concourse.bass_utils import run_bass_kernel_spmd

F32 = mybir.dt.float32
BF16 = mybir.dt.bfloat16
AF = mybir.ActivationFunctionType
ALU = mybir.AluOpType
AX = mybir.AxisListType

N, K, M, D, P = 128, 1024, 512, 128, 4096
NCORES = 8
NB = N // NCORES  # batches per core = 16


def _bcast_free(ap, reps):
    """Append a 0-stride innermost free dim: (.., F) -> (.., F, reps)."""
    return bass.AP(tensor=ap.tensor, offset=ap.offset,
                   ap=list(ap.ap) + [[0, reps]])


def _rep_free(ap, reps):
    """Prepend a 0-stride outer free dim: (p, F) -> (p, reps, F)."""
    return bass.AP(tensor=ap.tensor, offset=ap.offset,
                   ap=[ap.ap[0]] + [[0, reps]] + list(ap.ap[1:]))


def build_graph(cap: int, T: float) -> bass.Bass:
    """One-core SPMD graph; cap = padded pairs per batch; T = temperature."""
    C = NB * cap          # padded pairs per core
    NT = C // 128         # 128-pair tiles
    NBP = NB // 2         # batch-pairs (two batches share a 128-partition tile)
    assert cap == 64 and NT == NBP == 8

    nc = bacc.Bacc(None, target_bir_lowering=False, debug=False)

    xt = nc.declare_dram_parameter("xt", [D, NB * K], BF16, isOutput=False)
    ancT = nc.declare_dram_parameter("ancT", [D, C], BF16, isOutput=False)
    posT = nc.declare_dram_parameter("posT", [D, C], BF16, isOutput=False)
    # tiled-natural layouts: [128, ntiles*D], tile t = rows [t*128,(t+1)*128)
    anc = nc.declare_dram_parameter("anc", [128, NT * D], BF16, isOutput=False)
    pos = nc.declare_dram_parameter("pos", [128, NT * D], BF16, isOutput=False)
    rng = nc.declare_dram_parameter("rng", [128, 2 * NT * D], BF16, isOutput=False)
    phr = nc.declare_dram_parameter("phr", [128, (M // 128) * D], BF16, isOutput=False)
    vld = nc.declare_dram_parameter("vld", [128, NT], F32, isOutput=False)
    eye = nc.declare_dram_parameter("eye", [128, 128], BF16, isOutput=False)
    out = nc.declare_dram_parameter("out", [2 * NT, 1], F32, isOutput=True)

    NPH = M // 128  # phrase tiles = 4
    # nsml column map: [0:NT]=anc, [NT:2NT]=pos, [2NT:4NT]=rng, [4NT:4NT+NPH]=phr
    IA, IP, IR, IH = 0, NT, 2 * NT, 4 * NT
    NS = 4 * NT + NPH

    with TileContext(nc) as tc:
        with (
            tc.tile_pool(name="big", bufs=1) as big,
            tc.tile_pool(name="rowp", bufs=2) as rowp,
            tc.tile_pool(name="work", bufs=4) as work,
            tc.tile_pool(name="small", bufs=8) as small,
            tc.tile_pool(name="pn2", bufs=1, space="PSUM") as pn2,
            tc.tile_pool(name="prp", bufs=2, space="PSUM") as prp,
            tc.tile_pool(name="pce", bufs=1, space="PSUM") as pce,
            tc.tile_pool(name="ptp", bufs=1, space="PSUM") as ptp,
        ):
            # ---- constants / small inputs ----
            eye_sb = big.tile([128, 128], BF16, tag="eye")
            nc.sync.dma_start(out=eye_sb, in_=eye[:, :])
            vld_sb = big.tile([128, NT], F32, tag="vld")
            nc.sync.dma_start(out=vld_sb, in_=vld[:, :])
            ones64 = big.tile([128, 64], BF16, tag="ones64")
            nc.vector.memset(ones64, 1.0)
            ones_col = big.tile([128, 1], F32, tag="ones")
            nc.vector.memset(ones_col, 1.0)

            # warm the ACT table (rsqrt set) while DMAs run
            warm = small.tile([128, 8], F32, tag="warm")
            nc.vector.memset(warm, 1.0)
            warm2 = small.tile([128, 8], F32, tag="warm2")
            nc.scalar.activation(warm2, warm, AF.Abs_reciprocal_sqrt)

            # persistent operand tiles (each one contiguous DMA); xt chunks
            # first so the square pipeline starts as early as possible
            xt_sb = big.tile([128, NB * K], BF16, tag="xt")
            for q in range(8):
                nc.sync.dma_start(out=xt_sb[:, q * 2048:(q + 1) * 2048],
                                  in_=xt[:, q * 2048:(q + 1) * 2048])
            phr_sb = big.tile([128, NPH * D], BF16, tag="phr")
            nc.sync.dma_start(out=phr_sb, in_=phr[:, :])
            ancT_sb = big.tile([128, C], BF16, tag="ancT")
            nc.sync.dma_start(out=ancT_sb, in_=ancT[:, :])
            posT_sb = big.tile([128, C], BF16, tag="posT")
            nc.sync.dma_start(out=posT_sb, in_=posT[:, :])
            anc_sb = big.tile([128, NT * D], BF16, tag="anc")
            nc.sync.dma_start(out=anc_sb, in_=anc[:, :])
            pos_sb = big.tile([128, NT * D], BF16, tag="pos")
            nc.sync.dma_start(out=pos_sb, in_=pos[:, :])
            rng_sb = big.tile([128, 2 * NT * D], BF16, tag="rng")
            nc.sync.dma_start(out=rng_sb, in_=rng[:, :])
            phatT = big.tile([128, M], BF16, tag="phatT")

            # results
            t8_all = big.tile([128, NT * 8], F32, tag="t8")
            sposr = big.tile([128, NT], F32, tag="sposr")
            srndr = big.tile([128, 2 * NT], F32, tag="srndr")
            sumexp = big.tile([128, NT], F32, tag="sumexp")
            stat = big.tile([128, 2 * NT], F32, tag="stat")
            nsml = big.tile([128, NS], F32, tag="nsml")
            rinv = big.tile([128, NS], F32, tag="rinv")

            def sq_dot(src_sb, t, col):
                j = work.tile([128, D], BF16, tag="jd")
                nc.vector.scalar_tensor_tensor(
                    j, src_sb[:, t * D:(t + 1) * D], 1.0,
                    src_sb[:, t * D:(t + 1) * D],
                    op0=ALU.mult, op1=ALU.mult, accum_out=nsml[:, col:col + 1])

            # ---- phrase-hat + its transpose first (feeds CE matmuls) ----
            for t in range(NPH):
                sq_dot(phr_sb, t, IH + t)
            nc.scalar.activation(rinv[:, IH:IH + NPH], nsml[:, IH:IH + NPH],
                                 AF.Abs_reciprocal_sqrt)
            for t in range(NPH):
                ph = work.tile([128, D], BF16, tag="ph")
                nc.vector.tensor_scalar_mul(
                    ph, phr_sb[:, t * D:(t + 1) * D], rinv[:, IH + t:IH + t + 1])
                ps = ptp.tile([128, 512], BF16, tag="ptr")
                nc.tensor.transpose(ps[:, :128], ph, eye_sb)
                nc.scalar.copy(phatT[:, t * 128:(t + 1) * 128], ps[:, :128])

            # ---- phase B: input norms for every batch-pair (ACT stays on the
            # rsqrt table set throughout; its Square ops are grouped before
            # the first rsqrt so at most one extra table load can occur) ----
            sq_all = big.tile([128, NB * K], BF16, tag="sqall")
            rnb_all = big.tile([128, NBP * 1024], BF16, tag="rnball")
            for bp in range(NBP):
                sq = sq_all[:, bp * 2048:(bp + 1) * 2048]
                xsl = xt_sb[:, bp * 2048:(bp + 1) * 2048]
                if bp % 2 == 0:
                    nc.gpsimd.tensor_mul(sq, xsl, xsl)
                else:
                    nc.vector.tensor_mul(sq, xsl, xsl)

                # norms^2, replicated into the right partition halves
                n2 = pn2.tile([128, 1024], F32, tag="n2")
                for h in range(2):      # batch within pair
                    for g in range(2):  # k-half
                        nc.tensor.matmul(
                            n2[64 * h:64 * h + 64, g * 512:(g + 1) * 512],
                            ones64, sq[:, h * 1024 + g * 512:h * 1024 + (g + 1) * 512],
                            start=True, stop=True)
                nc.scalar.activation(rnb_all[:, bp * 1024:(bp + 1) * 1024],
                                     n2, AF.Abs_reciprocal_sqrt)

            # remaining small norms + their single rsqrt, plus the raw
            # s_pos / s_rand dots — all DVE work that fits in the B window
            for t in range(NT):
                sq_dot(anc_sb, t, IA + t)
            for t in range(NT):
                sq_dot(pos_sb, t, IP + t)
            for t in range(2 * NT):
                sq_dot(rng_sb, t, IR + t)
            nc.scalar.activation(rinv[:, 0:IH], nsml[:, 0:IH],
                                 AF.Abs_reciprocal_sqrt)
            for bp in range(NBP):
                jd = work.tile([128, D], BF16, tag="jp")
                nc.vector.scalar_tensor_tensor(
                    jd, anc_sb[:, bp * D:(bp + 1) * D], 1.0,
                    pos_sb[:, bp * D:(bp + 1) * D],
                    op0=ALU.mult, op1=ALU.mult,
                    accum_out=sposr[:, bp:bp + 1])
                for r in range(2):
                    jr = work.tile([128, D], BF16, tag="jr")
                    nc.vector.scalar_tensor_tensor(
                        jr, anc_sb[:, bp * D:(bp + 1) * D], 1.0,
                        rng_sb[:, (r * NT + bp) * D:(r * NT + bp + 1) * D],
                        op0=ALU.mult, op1=ALU.mult,
                        accum_out=srndr[:, r * NT + bp:r * NT + bp + 1])

            # CE exp scale = T * rinv_pos; the bypass dep on the LAST rnb tile
            # pins every Exp after the last rsqrt (one table switch total)
            tspp = small.tile([128, NT], F32, tag="tspp")
            nc.vector.scalar_tensor_tensor(
                tspp, rinv[:, IP:IP + NT], float(T),
                rnb_all[:, (NBP - 1) * 1024:(NBP - 1) * 1024 + NT],
                op0=ALU.mult, op1=ALU.bypass)

            # ---- phase C: sim rows, top-8, dots, CE ----
            for bp in range(NBP):
                rnb = rnb_all[:, bp * 1024:(bp + 1) * 1024]
                # sim rows (raw anchors x raw inputs)
                rp = prp.tile([128, 1024], F32, tag="rp")
                for h in range(2):
                    b = 2 * bp + h
                    acols = ancT_sb[:, b * cap:(b + 1) * cap]
                    for g in range(2):
                        nc.tensor.matmul(
                            rp[64 * h:64 * h + 64, g * 512:(g + 1) * 512],
                            acols,
                            xt_sb[:, b * K + g * 512:b * K + (g + 1) * 512],
                            start=True, stop=True)

                # cos rows = rp * rinv_anc (per-partition) * rnb (tensor)
                rows_sc = rowp.tile([128, 1024], BF16, tag="rsc")
                nc.vector.scalar_tensor_tensor(
                    rows_sc, rp, rinv[:, IA + bp:IA + bp + 1], rnb,
                    op0=ALU.mult, op1=ALU.mult)
                nc.vector.max(t8_all[:, bp * 8:(bp + 1) * 8], rows_sc)

                # CE tile bp: logits = posT_raw^T @ phatT; exp scale folds
                # T * rinv_pos; accumulate sum-exp
                lg = pce.tile([128, 512], F32, tag="lg")
                nc.tensor.matmul(lg, posT_sb[:, bp * 128:(bp + 1) * 128],
                                 phatT, start=True, stop=True)
                je = work.tile([128, 512], BF16, tag="je")
                nc.scalar.activation(je, lg, AF.Exp,
                                     scale=tspp[:, bp:bp + 1],
                                     accum_out=sumexp[:, bp:bp + 1])

            # ---- finale ----
            rinva = rinv[:, IA:IA + NT]
            spos = small.tile([128, NT], F32, tag="spos")
            nc.vector.tensor_mul(spos, sposr, rinva)
            nc.vector.tensor_mul(spos, spos, rinv[:, IP:IP + NT])
            srnd = small.tile([128, 2 * NT], F32, tag="srnd")
            nc.vector.tensor_mul(srnd, srndr, rinv[:, IR:IR + 2 * NT])
            nc.vector.tensor_mul(srnd, srnd, _rep_free(rinva, 2))

            t83 = t8_all[:, :].rearrange("p (t e) -> p t e", e=8)
            u_all = big.tile([128, NT * 8], F32, tag="uall")
            u3 = u_all[:, :].rearrange("p (t e) -> p t e", e=8)
            nc.vector.scalar_tensor_tensor(
                u3, t83, 1.0, _bcast_free(spos[:, :], 8),
                op0=ALU.add, op1=ALU.subtract)
            nc.vector.tensor_scalar_max(u_all, u_all, 0.0)
            s4 = small.tile([128, NT], F32, tag="s4")
            nc.vector.tensor_reduce(s4, u3[:, :, 0:4], AX.X, ALU.add)
            w = small.tile([128, NT], F32, tag="w")
            u4th = u_all[:, 3:4]
            u4th = bass.AP(tensor=u4th.tensor, offset=u4th.offset,
                           ap=[u4th.ap[0], [8, NT]])
            nc.vector.tensor_scalar_max(w, u4th, 1.0)
            hard = small.tile([128, NT], F32, tag="hard")
            nc.vector.tensor_sub(hard, s4, w)
            ur = small.tile([128, 2 * NT], F32, tag="ur")
            ur3 = ur[:, :].rearrange("p (t r) -> p t r", r=2)
            nc.vector.scalar_tensor_tensor(
                ur3, srnd[:, :].rearrange("p (r t) -> p t r", r=2), 1.0,
                _bcast_free(spos[:, :], 2),
                op0=ALU.add, op1=ALU.subtract)
            nc.vector.tensor_scalar_max(ur, ur, 0.0)
            r2 = small.tile([128, NT], F32, tag="r2")
            nc.vector.tensor_reduce(r2, ur3, AX.X, ALU.add)
            tript = small.tile([128, NT], F32, tag="tript")
            nc.vector.tensor_add(tript, hard, r2)
            nc.vector.tensor_mul(stat[:, 0:NT], tript, vld_sb)

            lnse = small.tile([128, NT], F32, tag="lnse")
            nc.scalar.activation(lnse, sumexp, AF.Ln)
            tsp = small.tile([128, NT], F32, tag="tsp")
            nc.vector.tensor_scalar_mul(tsp, spos, float(T))
            cet = small.tile([128, NT], F32, tag="cet")
            nc.vector.tensor_sub(cet, lnse, tsp)
            nc.vector.tensor_mul(stat[:, NT:2 * NT], cet, vld_sb)

            # cross-partition reduction: out[j] = sum_p stat[p, j]
            pres = pce.tile([128, 512], F32, tag="lg")
            nc.tensor.matmul(
                pres[:2 * NT, :1], stat, ones_col, start=True, stop=True)
            res_sb = small.tile([2 * NT, 1], F32, tag="res")
            nc.vector.tensor_copy(res_sb, pres[:2 * NT, :1])
            nc.sync.dma_start(out=out[:, :], in_=res_sb[:, :])

    if not nc.is_finalized():
        nc.finalize()
    return nc


_CACHE = {}
_BF16 = mybir.dt.np(BF16)


def _tiled(a, ntiles):
    """[ntiles*128, D] -> [128, ntiles*D] device tile layout."""
    return np.ascontiguousarray(
        a.reshape(ntiles, 128, D).transpose(1, 0, 2).reshape(128, ntiles * D))


def _prep_core(c, cap, pe, ie, bi, mi, ki, rn, T):
    C = NB * cap
    NT = C // 128
    lo = NB * c
    sel = np.where((bi >= lo) & (bi < lo + NB))[0]
    # pad with unit vectors so normalization never divides by zero
    ancb = np.zeros((C, D), np.float32); ancb[:, 0] = 1.0
    posb = np.zeros((C, D), np.float32); posb[:, 0] = 1.0
    rngb = np.zeros((2 * C, D), np.float32); rngb[:, 0] = 1.0
    valid = np.zeros(C, np.float32)
    for n in range(NB):
        pb = sel[bi[sel] == lo + n]
        assert len(pb) <= cap
        s = n * cap
        ancb[s:s + len(pb)] = pe[mi[pb]]
        posb[s:s + len(pb)] = ie[bi[pb], ki[pb]]
        rngb[s:s + len(pb)] = ie[bi[pb], rn[pb, 0]]
        rngb[C + s:C + s + len(pb)] = ie[bi[pb], rn[pb, 1]]
        valid[s:s + len(pb)] = 1.0
    xt_c = np.ascontiguousarray(
        ie[lo:lo + NB].reshape(NB * K, D).T).astype(_BF16)
    ancb = ancb.astype(_BF16)
    posb = posb.astype(_BF16)
    vld_dev = np.ascontiguousarray(valid.reshape(NT, 128).T)
    return dict(
        xt=xt_c,
        ancT=np.ascontiguousarray(ancb.T),
        posT=np.ascontiguousarray(posb.T),
        anc=_tiled(ancb, NT), pos=_tiled(posb, NT),
        rng=_tiled(rngb.astype(_BF16), 2 * NT),
        phr=_tiled(pe.astype(_BF16), M // 128),
        vld=vld_dev,
        eye=np.eye(128, dtype=_BF16),
    )


def make_in_maps(inputs, cap=None):
    pe = np.asarray(inputs["phrase_embeddings"], np.float32)
    ie = np.asarray(inputs["input_embeddings"], np.float32)
    bi = np.asarray(inputs["batch_idxs"])
    mi = np.asarray(inputs["phrase_emb_idxs"])
    ki = np.asarray(inputs["input_emb_idxs"])
    rn = np.asarray(inputs["rand_neg_idx"])
    T = float(np.asarray(inputs["temperature"]))
    if cap is None:
        maxc = int(np.bincount(bi, minlength=N).max())
        cap = max(64, ((maxc + 63) // 64) * 64)
    return [
        _prep_core(c, cap, pe, ie, bi, mi, ki, rn, T) for c in range(NCORES)
    ], cap, T


def kernel(**inputs):
    in_maps, cap, T = make_in_maps(inputs)
    key = (cap, T)
    if key not in _CACHE:
        _CACHE[key] = build_graph(cap, T)
    nc = _CACHE[key]
    res = run_bass_kernel_spmd(nc, in_maps, core_ids=list(range(NCORES)))
    outs = np.stack([np.asarray(r["out"]).reshape(-1) for r in res.results])
    NT = NB * cap // 128
    trip = outs[:, :NT].sum() / (P * 5)
    ce = outs[:, NT:].sum() / P
    return np.float32(trip), np.float32(ce)



# revision 5
# speedup vs baseline: 1.7944x; 1.7944x over previous
"""Trainium2 Bass kernel for nn_AlignmentLoss (triplet + CE over phrase/input embeddings).

Sharding: batch dimension N=128 split 16 batches/core across 8 cores.  Each core
owns the positive pairs whose batch_idxs falls in its range (host buckets pairs,
padded to a fixed per-batch capacity cap=64; 2 batches share a 128-partition tile).

v5 design:
 - Host L2-normalizes phrase and input embeddings in f32 (exactly the
   reference's F.normalize preprocessing), so the device never computes
   norms: no squares, no ones-matmuls, no rsqrts, no row rescaling.
 - Sim-row and CE matmul operands ship as fp8 (e4m3): halves the big
   HBM transfer; cos rows come straight out of the PE into PSUM and are
   consumed by DVE Max8 directly from PSUM (no copies).
 - Triplet dot products (anchor*pos, anchor*rng) run as one GpSimd
   elementwise multiply (hidden under the sim pipeline) + one DVE
   grouped reduce.
 - CE: fp8 matmul logits -> ACT Exp(scale=T) with accum_out; Exp and Ln
   share one ACT table set so there are zero mid-kernel table loads.
 - DMA issues spread across Sync/Scalar/Pool sequencers so transfers
   start flowing immediately.
"""

import sys

for _p in ("/opt/trn_rl_repo", "/root/.axon_site/_ro/trn_rl_repo"):
    if _p not in sys.path:
        sys.path.append(_p)

import numpy as np

import concourse.bass as bass
import concourse.bacc as bacc
import concourse.mybir as mybir
from concourse.tile import TileContext
from concourse.bass_utils import run_bass_kernel_spmd

F32 = mybir.dt.float32
BF16 = mybir.dt.bfloat16
FP8 = mybir.dt.float8e4
AF = mybir.ActivationFunctionType
ALU = mybir.AluOpType
AX = mybir.AxisListType

N, K, M, D, P = 128, 1024, 512, 128, 4096
NCORES = 8
NB = N // NCORES  # batches per core = 16


def _ap(ap, dims):
    """Rebuild an AP with explicit [stride, count] free dims."""
    return bass.AP(tensor=ap.tensor, offset=ap.offset,
                   ap=[ap.ap[0]] + [list(d) for d in dims])


def build_graph(cap: int, T: float) -> bass.Bass:
    """One-core SPMD graph; cap = padded pairs per batch; T = temperature."""
    C = NB * cap          # padded pairs per core
    NT = C // 128         # 128-pair tiles
    BPT = 128 // cap      # batches per tile
    assert NT * 128 == C and BPT * cap == 128
    NCH = 4               # xt DMA chunks
    assert NT % NCH == 0

    nc = bacc.Bacc(None, target_bir_lowering=False, debug=False)

    xt = nc.declare_dram_parameter("xt", [D, NB * K], FP8, isOutput=False)
    ancT = nc.declare_dram_parameter("ancT", [D, C], FP8, isOutput=False)
    posT = nc.declare_dram_parameter("posT", [D, C], FP8, isOutput=False)
    phrT = nc.declare_dram_parameter("phrT", [D, M], FP8, isOutput=False)
    anc = nc.declare_dram_parameter("anc", [128, NT * D], BF16, isOutput=False)
    neg3 = nc.declare_dram_parameter("neg3", [128, NT * 3 * D], BF16, isOutput=False)
    vld2 = nc.declare_dram_parameter("vld2", [128, 2 * NT], F32, isOutput=False)
    out = nc.declare_dram_parameter("out", [2 * NT, 1], F32, isOutput=True)

    KCOL = NB * K // NCH  # xt columns per DMA chunk

    with TileContext(nc) as tc:
        with (
            tc.tile_pool(name="big", bufs=1) as big,
            tc.tile_pool(name="work", bufs=2) as work,
            tc.tile_pool(name="small", bufs=8) as small,
            tc.tile_pool(name="prow", bufs=3, space="PSUM") as prow,
            tc.tile_pool(name="pce", bufs=2, space="PSUM") as pce,
        ):
            # ---- persistent tiles ----
            xt_sb = big.tile([128, NB * K], FP8, tag="xt")
            ancT_sb = big.tile([128, C], FP8, tag="ancT")
            posT_sb = big.tile([128, C], FP8, tag="posT")
            phrT_sb = big.tile([128, M], FP8, tag="phrT")
            anc_sb = big.tile([128, NT * D], BF16, tag="anc")
            neg3_sb = big.tile([128, NT * 3 * D], BF16, tag="neg3")
            vld2_sb = big.tile([128, 2 * NT], F32, tag="vld2")
            prod = big.tile([128, NT * 3 * D], BF16, tag="prod")
            t8_all = big.tile([128, NT * 8], F32, tag="t8")
            sdots = big.tile([128, NT * 3], F32, tag="sdots")
            sumexp = big.tile([128, NT], F32, tag="sumexp")
            stat = big.tile([128, 2 * NT], F32, tag="stat")
            ones_col = big.tile([128, 1], F32, tag="ones")

            # ---- DMA issues, spread across sequencers ----
            # sync: ancT + first half of xt (feeds sim matmuls)
            nc.sync.dma_start(out=ancT_sb, in_=ancT[:, :])
            for q in range(NCH // 2):
                nc.sync.dma_start(out=xt_sb[:, q * KCOL:(q + 1) * KCOL],
                                  in_=xt[:, q * KCOL:(q + 1) * KCOL])
            # scalar: CE operands first (CE matmuls warm the PE), then xt tail
            nc.scalar.dma_start(out=posT_sb, in_=posT[:, :])
            nc.scalar.dma_start(out=phrT_sb, in_=phrT[:, :])
            for q in range(NCH // 2, NCH):
                nc.scalar.dma_start(out=xt_sb[:, q * KCOL:(q + 1) * KCOL],
                                    in_=xt[:, q * KCOL:(q + 1) * KCOL])
            # pool: triplet dot operands + validity
            nc.gpsimd.dma_start(out=anc_sb, in_=anc[:, :])
            nc.gpsimd.dma_start(out=neg3_sb, in_=neg3[:, :])
            nc.gpsimd.dma_start(out=vld2_sb, in_=vld2[:, :])
            nc.vector.memset(ones_col, 1.0)

            # ---- GpSimd: anchor * [pos|rng0|rng1] elementwise (one op) ----
            anc_b3 = _ap(anc_sb[:, :], [[D, NT], [0, 3], [1, D]])
            nc.gpsimd.tensor_mul(
                prod[:, :].rearrange("p (t r d) -> p t r d", r=3, d=D),
                anc_b3,
                neg3_sb[:, :].rearrange("p (t r d) -> p t r d", r=3, d=D))

            def ce_mm(t):
                lg = pce.tile([128, 512], F32, tag="lg")
                nc.tensor.matmul(lg, posT_sb[:, t * 128:(t + 1) * 128],
                                 phrT_sb, start=True, stop=True)
                je = work.tile([128, 512], BF16, tag="je")
                nc.scalar.activation(je, lg, AF.Exp, scale=float(T),
                                     accum_out=sumexp[:, t:t + 1])

            def sim_mm(t):
                rp = prow.tile([128, 1024], F32, tag="rp")
                for h in range(BPT):
                    b = BPT * t + h
                    acols = ancT_sb[:, b * cap:(b + 1) * cap]
                    for g in range(K // 512):
                        nc.tensor.matmul(
                            rp[cap * h:cap * (h + 1), g * 512:(g + 1) * 512],
                            acols,
                            xt_sb[:, b * K + g * 512:b * K + (g + 1) * 512],
                            start=True, stop=True)
                nc.vector.max(t8_all[:, t * 8:(t + 1) * 8], rp)

            # PE order: a few CE matmuls first (tiny DMA deps; they warm the
            # p-state), then interleave sims as xt chunks land.
            ce_mm(0); ce_mm(1); ce_mm(2)
            nxt = 3
            for t in range(NT):
                sim_mm(t)
                if nxt < NT:
                    ce_mm(nxt)
                    nxt += 1
            # ---- finale ----
            # DVE grouped reduce of the GpSimd triplet-dot products
            nc.vector.tensor_reduce(
                sdots[:, :],
                prod[:, :].rearrange("p (g x) -> p g x", x=D),
                AX.X, ALU.add)

            spos = _ap(sdots[:, :], [[3, NT]])                      # [128, NT]
            spos4 = _ap(sdots[:, :], [[3, NT], [0, 4]])
            t84 = _ap(t8_all[:, :], [[8, NT], [1, 4]])
            u4t = small.tile([128, NT * 4], F32, tag="u4t")
            nc.vector.scalar_tensor_tensor(
                u4t[:, :].rearrange("p (t e) -> p t e", e=4), t84, 1.0, spos4,
                op0=ALU.add, op1=ALU.subtract)
            nc.vector.tensor_scalar_max(u4t, u4t, 0.0)
            s4 = small.tile([128, NT], F32, tag="s4")
            nc.vector.tensor_reduce(
                s4, u4t[:, :].rearrange("p (t e) -> p t e", e=4), AX.X, ALU.add)
            w = small.tile([128, NT], F32, tag="w")
            u4th = _ap(u4t[:, 3:4], [[4, NT]])
            nc.vector.tensor_scalar_max(w, u4th, 1.0)

            srnd = _ap(sdots[:, 1:2], [[3, NT], [1, 2]])
            spos2 = _ap(sdots[:, :], [[3, NT], [0, 2]])
            ur = small.tile([128, NT * 2], F32, tag="ur")
            nc.vector.scalar_tensor_tensor(
                ur[:, :].rearrange("p (t e) -> p t e", e=2), srnd, 1.0, spos2,
                op0=ALU.add, op1=ALU.subtract)
            nc.vector.tensor_scalar_max(ur, ur, 0.0)
            r2 = small.tile([128, NT], F32, tag="r2")
            nc.vector.tensor_reduce(
                r2, ur[:, :].rearrange("p (t e) -> p t e", e=2), AX.X, ALU.add)

            hard = small.tile([128, NT], F32, tag="hard")
            nc.vector.tensor_sub(hard, s4, w)
            nc.vector.tensor_add(stat[:, 0:NT], hard, r2)

            lnse = small.tile([128, NT], F32, tag="lnse")
            nc.scalar.activation(lnse, sumexp, AF.Ln)
            # cet = lnse - T*spos  ==  (spos * -T) + lnse
            nc.vector.scalar_tensor_tensor(
                stat[:, NT:2 * NT], spos, -float(T), lnse,
                op0=ALU.mult, op1=ALU.add)
            nc.vector.tensor_mul(stat, stat, vld2_sb)

            # cross-partition reduction: out[j] = sum_p stat[p, j]
            pres = pce.tile([128, 512], F32, tag="lg")
            nc.tensor.matmul(
                pres[:2 * NT, :1], stat, ones_col, start=True, stop=True)
            res_sb = small.tile([2 * NT, 1], F32, tag="res")
            nc.vector.tensor_copy(res_sb, pres[:2 * NT, :1])
            nc.sync.dma_start(out=out[:, :], in_=res_sb[:, :])

    if not nc.is_finalized():
        nc.finalize()
    return nc


_CACHE = {}
_BF16 = mybir.dt.np(BF16)
_FP8 = mybir.dt.np(FP8)


def _l2n(x):
    return x / np.maximum(np.linalg.norm(x, axis=-1, keepdims=True), 1e-12)


def _tiled(a, ntiles, width):
    """[ntiles*128, width] -> [128, ntiles*width] device tile layout."""
    return np.ascontiguousarray(
        a.reshape(ntiles, 128, width).transpose(1, 0, 2).reshape(128, ntiles * width))


def _prep_core(c, cap, pe, ie, bi, mi, ki, rn):
    """pe/ie are pre-normalized f32."""
    C = NB * cap
    NT = C // 128
    lo = NB * c
    sel = np.where((bi >= lo) & (bi < lo + NB))[0]
    # pad with unit vectors (already normalized)
    ancb = np.zeros((C, D), np.float32); ancb[:, 0] = 1.0
    posb = np.zeros((C, D), np.float32); posb[:, 0] = 1.0
    rngb = np.zeros((C, 2, D), np.float32); rngb[:, :, 0] = 1.0
    valid = np.zeros(C, np.float32)
    for n in range(NB):
        pb = sel[bi[sel] == lo + n]
        assert len(pb) <= cap
        s = n * cap
        ancb[s:s + len(pb)] = pe[mi[pb]]
        posb[s:s + len(pb)] = ie[bi[pb], ki[pb]]
        rngb[s:s + len(pb), 0] = ie[bi[pb], rn[pb, 0]]
        rngb[s:s + len(pb), 1] = ie[bi[pb], rn[pb, 1]]
        valid[s:s + len(pb)] = 1.0
    xt_c = np.ascontiguousarray(
        ie[lo:lo + NB].reshape(NB * K, D).T).astype(_FP8)
    neg = np.concatenate([posb[:, None, :], rngb], axis=1)  # [C, 3, D]
    vt = np.ascontiguousarray(valid.reshape(NT, 128).T)
    return dict(
        xt=xt_c,
        ancT=np.ascontiguousarray(ancb.T).astype(_FP8),
        posT=np.ascontiguousarray(posb.T).astype(_FP8),
        phrT=np.ascontiguousarray(pe.T).astype(_FP8),
        anc=_tiled(ancb.astype(_BF16), NT, D),
        neg3=_tiled(neg.reshape(C, 3 * D).astype(_BF16), NT, 3 * D),
        vld2=np.ascontiguousarray(np.concatenate([vt, vt], axis=1)),
    )


def make_in_maps(inputs, cap=None):
    pe = _l2n(np.asarray(inputs["phrase_embeddings"], np.float32))
    ie = _l2n(np.asarray(inputs["input_embeddings"], np.float32))
    bi = np.asarray(inputs["batch_idxs"])
    mi = np.asarray(inputs["phrase_emb_idxs"])
    ki = np.asarray(inputs["input_emb_idxs"])
    rn = np.asarray(inputs["rand_neg_idx"])
    T = float(np.asarray(inputs["temperature"]))
    if cap is None:
        maxc = int(np.bincount(bi, minlength=N).max())
        cap = max(64, ((maxc + 63) // 64) * 64)
    return [
        _prep_core(c, cap, pe, ie, bi, mi, ki, rn) for c in range(NCORES)
    ], cap, T


def kernel(**inputs):
    in_maps, cap, T = make_in_maps(inputs)
    key = (cap, T)
    if key not in _CACHE:
        _CACHE[key] = build_graph(cap, T)
    nc = _CACHE[key]
    res = run_bass_kernel_spmd(nc, in_maps, core_ids=list(range(NCORES)))
    outs = np.stack([np.asarray(r["out"]).reshape(-1) for r in res.results])
    NT = NB * cap // 128
    trip = outs[:, :NT].sum() / (P * 5)
    ce = outs[:, NT:].sum() / P
    return np.float32(trip), np.float32(ce)
